# revision 2
# baseline (speedup 1.0000x reference)
"""Expert-parallel MoE FFN kernel for 8 Trainium2 NeuronCores.

Math (per expert e): out = gelu(x_e @ w1_e + b1_e) @ w2_e + b2_e
  x: [B=2, E=8, N=1024, D=1024], w1: [E, D, F=4096], b1: [E, F],
  w2: [E, F, D], b2: [E, D]  ->  out: [B, E, N, D]

Sharding: one expert per core (the e axis), outputs gathered on host.

Strategy (all matmuls bf16 at the PE streaming floor, fp32 accumulate):
  - Host staging casts w1/w2 to bf16 and uploads x already transposed
    (xT [d, tok], bf16), so the kernel needs NO on-chip transposes.
  - Per 1024-token block:
      mm1: psum[f,tok] += w1[d,f].T @ xT[d,tok]   (f chunks of 128,
           tok halves of 512, accumulated over all d)
      ACT applies exact Gelu with per-partition bias b1[f] moving
      PSUM -> SBUF hT [f, tok] (bf16).
      mm2: psum[tok,d] += hT[f,tok].T @ w2[f,d]   (accumulated over all
           f, d halves of 512)
      DVE adds broadcast b2; stores go out in natural [tok, d] layout.
  - bf16 weights halve HBM traffic; the 1024-token block means w1/w2
    are streamed only twice per invocation, so weight DMA never stalls
    the PE (keeps HAM at full clock).
  - Sustained bf16 MM spacing is ~216ns (512-col moving operand), vs
    ~227ns for fp32r whose 4-byte weight load does not fully hide.
Input loads ride the SP HWDGE ring; output stores ride the ACT ring so
weight prefetch is never head-of-line blocked behind stores.
"""

import sys

for _p in ("/opt/trn_rl_repo", "/opt/pypackages"):
    if _p not in sys.path:
        sys.path.append(_p)

import numpy as np
import ml_dtypes

B, E, N, D, F = 2, 8, 1024, 1024, 4096
TOK = B * N  # tokens per expert
TB = 1024  # token block
NBLK = TOK // TB
nD = D // 128
nF = F // 128
nTS = TB // 128

_CACHE: dict = {}


def _build(reps: int = 1):
    import concourse.bacc as bacc
    import concourse.bass as bass
    import concourse.tile as tile
    from concourse import mybir

    F32 = mybir.dt.float32
    BF16 = mybir.dt.bfloat16
    GELU = mybir.ActivationFunctionType.Gelu
    ADD = mybir.AluOpType.add

    nc = bacc.Bacc("TRN2", target_bir_lowering=False, debug=False, num_devices=8)

    xT = nc.dram_tensor("xT", [D, TOK], BF16, kind="ExternalInput").ap()
    w1 = nc.dram_tensor("w1", [D, F], BF16, kind="ExternalInput").ap()
    b1 = nc.dram_tensor("b1", [F], F32, kind="ExternalInput").ap()
    w2 = nc.dram_tensor("w2", [F, D], BF16, kind="ExternalInput").ap()
    b2 = nc.dram_tensor("b2", [D], F32, kind="ExternalInput").ap()
    out = nc.dram_tensor("out", [TOK, D], F32, kind="ExternalOutput").ap()

    # multi-dim views for coalesced DMAs
    xT3 = xT.rearrange("(dc p) (blk t) -> dc p blk t", p=128, t=TB)
    w1_4 = w1.rearrange("(dc p) (fg f) -> dc p fg f", p=128, f=512)
    w2_4 = w2.rearrange("(fq fc p) (dh c) -> fq fc p dh c", fc=4, p=128, c=512)
    out4 = out.rearrange("(blk ts p) (dh c) -> blk ts p dh c", ts=nTS, p=128, c=512)

    with tile.TileContext(nc) as tc:
        with (
            tc.tile_pool(name="consts", bufs=1) as consts,
            tc.tile_pool(name="xTp", bufs=1) as xTp,
            tc.tile_pool(name="hTp", bufs=1) as hTp,
            tc.tile_pool(name="w1p", bufs=3) as w1p,
            tc.tile_pool(name="w2p", bufs=6) as w2p,
            tc.tile_pool(name="op", bufs=2) as op,
            tc.tile_pool(name="ps", bufs=8, space="PSUM") as ps,
        ):
            b1_t = consts.tile([128, nF], F32, tag="b1")
            nc.sync.dma_start(out=b1_t, in_=b1.rearrange("(c p) -> p c", p=128))
            b2_t = consts.tile([128, D], F32, tag="b2")
            nc.gpsimd.dma_start(
                out=b2_t,
                in_=bass.AP(tensor=b2.tensor, offset=b2.offset, ap=[[0, 128], [1, D]]),
            )

            for blk in range(NBLK * reps):
                blk = blk % NBLK

                # --- load xT block [d, tok] (bf16, pre-transposed on host) ---
                xt = xTp.tile([128, nD, TB], BF16, tag="xT")
                nc.sync.dma_start(
                    out=xt, in_=xT3[:, :, blk].rearrange("dc p t -> p dc t")
                )

                # --- mm1 + gelu: hT [f, tok] (bf16) ---
                hT_t = hTp.tile([128, nF, TB], BF16, tag="hT")
                for fg in range(nF // 4):  # f slabs of 512
                    wt = w1p.tile([128, nD, 512], BF16, tag="w1")
                    nc.sync.dma_start(
                        out=wt, in_=w1_4[:, :, fg].rearrange("dc p f -> p dc f")
                    )
                    for fc4 in range(4):
                        fc = fg * 4 + fc4
                        for th in range(TB // 512):
                            ph = ps.tile(
                                [128, 512], F32, tag="ps", name=f"ph_{blk}_{fc}_{th}"
                            )
                            for dc in range(nD):
                                nc.tensor.matmul(
                                    ph,
                                    wt[:, dc, fc4 * 128 : (fc4 + 1) * 128],
                                    xt[:, dc, th * 512 : (th + 1) * 512],
                                    start=(dc == 0),
                                    stop=(dc == nD - 1),
                                )
                            nc.scalar.activation(
                                hT_t[:, fc, th * 512 : (th + 1) * 512],
                                ph,
                                GELU,
                                bias=b1_t[:, fc : fc + 1],
                                scale=1.0,
                            )

                # --- mm2 + b2: out [tok, d] ---
                for dh in range(D // 512):
                    pos = [
                        ps.tile([128, 512], F32, tag="ps", name=f"po_{blk}_{dh}_{i}")
                        for i in range(nTS)
                    ]
                    for fq in range(nF // 4):  # f chunks of 4x128
                        wt2 = w2p.tile([128, 4, 512], BF16, tag="w2")
                        nc.sync.dma_start(
                            out=wt2, in_=w2_4[fq, :, :, dh].rearrange("fc p c -> p fc c")
                        )
                        for fci in range(4):
                            fc = fq * 4 + fci
                            for ts in range(nTS):
                                nc.tensor.matmul(
                                    pos[ts],
                                    hT_t[:, fc, ts * 128 : (ts + 1) * 128],
                                    wt2[:, fci, :],
                                    start=(fc == 0),
                                    stop=(fc == nF - 1),
                                )
                    for tsg in range(nTS // 4):
                        ot = op.tile([128, 4, 512], F32, tag="o")
                        for i in range(4):
                            nc.vector.tensor_tensor(
                                out=ot[:, i, :],
                                in0=pos[tsg * 4 + i],
                                in1=b2_t[:, dh * 512 : (dh + 1) * 512],
                                op=ADD,
                            )
                        nc.scalar.dma_start(
                            out=out4[blk, tsg * 4 : (tsg + 1) * 4, :, dh].rearrange(
                                "ts p c -> p ts c"
                            ),
                            in_=ot,
                        )

    nc.compile()
    return nc


def _get_nc(reps: int = 1):
    key = f"nc{reps}"
    if key not in _CACHE:
        _CACHE[key] = _build(reps)
    return _CACHE[key]


def prep_in_maps(inputs: dict) -> list:
    """Host-side staging: cast weights/activations to bf16, pre-transpose x."""
    x = np.asarray(inputs["x"], dtype=np.float32)
    w1 = np.asarray(inputs["w1"], dtype=np.float32)
    b1 = np.asarray(inputs["b1"], dtype=np.float32)
    w2 = np.asarray(inputs["w2"], dtype=np.float32)
    b2 = np.asarray(inputs["b2"], dtype=np.float32)
    in_maps = []
    for e in range(E):
        xT = np.ascontiguousarray(x[:, e].reshape(TOK, D).T).astype(ml_dtypes.bfloat16)
        in_maps.append(
            {
                "xT": xT,
                "w1": np.ascontiguousarray(w1[e]).astype(ml_dtypes.bfloat16),
                "b1": np.ascontiguousarray(b1[e]),
                "w2": np.ascontiguousarray(w2[e]).astype(ml_dtypes.bfloat16),
                "b2": np.ascontiguousarray(b2[e]),
            }
        )
    return in_maps


def kernel(x, w1, b1, w2, b2):
    from concourse.bass_utils import run_bass_kernel_spmd

    nc = _get_nc()
    in_maps = prep_in_maps({"x": x, "w1": w1, "b1": b1, "w2": w2, "b2": b2})
    res = run_bass_kernel_spmd(nc, in_maps, list(range(E)))
    out = np.empty((B, E, N, D), np.float32)
    for e in range(E):
        out[:, e] = res.results[e]["out"].reshape(B, N, D)
    return out


# revision 3
# speedup vs baseline: 2.2783x; 2.2783x over previous
"""Expert-parallel MoE FFN kernel for 8 Trainium2 NeuronCores.

Math (per expert e): out = gelu(x_e @ w1_e + b1_e) @ w2_e + b2_e
  x: [B=2, E=8, N=1024, D=1024], w1: [E, D, F=4096], b1: [E, F],
  w2: [E, F, D], b2: [E, D]  ->  out: [B, E, N, D]

Sharding: one expert per core (the e axis), outputs gathered on host.

Strategy (all matmuls bf16 at the PE streaming floor, fp32 accumulate):
  - Host staging casts w1/w2 to bf16 and uploads x already transposed
    (xT [d, tok], bf16), so the kernel needs NO on-chip transposes.
  - Per 1024-token block:
      mm1: psum[f,tok] += w1[d,f].T @ xT[d,tok]   (f chunks of 128,
           tok halves of 512, accumulated over all d)
      ACT applies exact Gelu with per-partition bias b1[f] moving
      PSUM -> SBUF hT [f, tok] (bf16).
      mm2: psum[tok,d] += hT[f,tok].T @ w2[f,d]   (accumulated over all
           f, d halves of 512)
      DVE adds broadcast b2; stores go out in natural [tok, d] layout.
  - bf16 weights halve HBM traffic; the 1024-token block means w1/w2
    are streamed only twice per invocation, so weight DMA never stalls
    the PE (keeps HAM at full clock).
  - Sustained bf16 MM spacing is ~216ns (512-col moving operand), vs
    ~227ns for fp32r whose 4-byte weight load does not fully hide.
Input loads ride the SP HWDGE ring; output stores ride the ACT ring so
weight prefetch is never head-of-line blocked behind stores.
"""

import sys

for _p in ("/opt/trn_rl_repo", "/opt/pypackages"):
    if _p not in sys.path:
        sys.path.append(_p)

import numpy as np
import ml_dtypes

B, E, N, D, F = 2, 8, 1024, 1024, 4096
TOK = B * N  # tokens per expert
TB = 1024  # token block
NBLK = TOK // TB
nD = D // 128
nF = F // 128
nTS = TB // 128

_CACHE: dict = {}


def _build(reps: int = 1):
    import concourse.bacc as bacc
    import concourse.bass as bass
    import concourse.tile as tile
    from concourse import mybir

    F32 = mybir.dt.float32
    BF16 = mybir.dt.bfloat16
    GELU = mybir.ActivationFunctionType.Gelu
    ADD = mybir.AluOpType.add

    nc = bacc.Bacc("TRN2", target_bir_lowering=False, debug=False, num_devices=8)

    xT = nc.dram_tensor("xT", [D, TOK], BF16, kind="ExternalInput").ap()
    w1 = nc.dram_tensor("w1", [D, F], BF16, kind="ExternalInput").ap()
    b1 = nc.dram_tensor("b1", [F], F32, kind="ExternalInput").ap()
    w2 = nc.dram_tensor("w2", [F, D], BF16, kind="ExternalInput").ap()
    b2 = nc.dram_tensor("b2", [D], F32, kind="ExternalInput").ap()
    out = nc.dram_tensor("out", [TOK, D], F32, kind="ExternalOutput").ap()

    # multi-dim views for coalesced DMAs
    xT3 = xT.rearrange("(dc p) (blk t) -> dc p blk t", p=128, t=TB)
    w2_4 = w2.rearrange("(fq fc p) (dh c) -> fq fc p dh c", fc=4, p=128, c=512)
    out4 = out.rearrange("(blk ts p) (dh c) -> blk ts p dh c", ts=nTS, p=128, c=512)

    with tile.TileContext(nc) as tc:
        with (
            tc.tile_pool(name="consts", bufs=1) as consts,
            tc.tile_pool(name="xTp", bufs=1) as xTp,
            tc.tile_pool(name="hTp", bufs=1) as hTp,
            tc.tile_pool(name="w2p", bufs=6) as w2p,
            tc.tile_pool(name="op", bufs=2) as op,
            tc.tile_pool(name="ps", bufs=8, space="PSUM") as ps,
        ):
            b1_t = consts.tile([128, nF], F32, tag="b1")
            nc.sync.dma_start(out=b1_t, in_=b1.rearrange("(c p) -> p c", p=128))
            b2_t = consts.tile([128, D], F32, tag="b2")
            nc.gpsimd.dma_start(
                out=b2_t,
                in_=bass.AP(tensor=b2.tensor, offset=b2.offset, ap=[[0, 128], [1, D]]),
            )
            # w1 is small enough in bf16 to stay SBUF-resident: load once per
            # dispatch instead of streaming 8MB per block.
            w1r = consts.tile([128, nD, F], BF16, tag="w1r")
            nc.sync.dma_start(out=w1r, in_=w1.rearrange("(dc p) f -> p dc f", p=128))

            for blk in range(NBLK * reps):
                blk = blk % NBLK

                # --- load xT block [d, tok] (bf16, pre-transposed on host) ---
                xt = xTp.tile([128, nD, TB], BF16, tag="xT")
                nc.sync.dma_start(
                    out=xt, in_=xT3[:, :, blk].rearrange("dc p t -> p dc t")
                )

                # --- mm1 + gelu: hT [f, tok] (bf16) ---
                hT_t = hTp.tile([128, nF, TB], BF16, tag="hT")
                for fc in range(nF):
                    for th in range(TB // 512):
                        ph = ps.tile(
                            [128, 512], F32, tag="ps", name=f"ph_{blk}_{fc}_{th}"
                        )
                        for dc in range(nD):
                            nc.tensor.matmul(
                                ph,
                                w1r[:, dc, fc * 128 : (fc + 1) * 128],
                                xt[:, dc, th * 512 : (th + 1) * 512],
                                start=(dc == 0),
                                stop=(dc == nD - 1),
                            )
                        nc.scalar.activation(
                            hT_t[:, fc, th * 512 : (th + 1) * 512],
                            ph,
                            GELU,
                            bias=b1_t[:, fc : fc + 1],
                            scale=1.0,
                        )

                # --- mm2 + b2: out [tok, d] ---
                for dh in range(D // 512):
                    pos = [
                        ps.tile([128, 512], F32, tag="ps", name=f"po_{blk}_{dh}_{i}")
                        for i in range(nTS)
                    ]
                    for fq in range(nF // 4):  # f chunks of 4x128
                        wt2 = w2p.tile([128, 4, 512], BF16, tag="w2")
                        nc.sync.dma_start(
                            out=wt2, in_=w2_4[fq, :, :, dh].rearrange("fc p c -> p fc c")
                        )
                        for fci in range(4):
                            fc = fq * 4 + fci
                            for ts in range(nTS):
                                nc.tensor.matmul(
                                    pos[ts],
                                    hT_t[:, fc, ts * 128 : (ts + 1) * 128],
                                    wt2[:, fci, :],
                                    start=(fc == 0),
                                    stop=(fc == nF - 1),
                                )
                    for tsg in range(nTS // 4):
                        ot = op.tile([128, 4, 512], F32, tag="o")
                        for i in range(4):
                            nc.vector.tensor_tensor(
                                out=ot[:, i, :],
                                in0=pos[tsg * 4 + i],
                                in1=b2_t[:, dh * 512 : (dh + 1) * 512],
                                op=ADD,
                            )
                        nc.scalar.dma_start(
                            out=out4[blk, tsg * 4 : (tsg + 1) * 4, :, dh].rearrange(
                                "ts p c -> p ts c"
                            ),
                            in_=ot,
                        )

    nc.compile()
    return nc


def _get_nc(reps: int = 1):
    key = f"nc{reps}"
    if key not in _CACHE:
        _CACHE[key] = _build(reps)
    return _CACHE[key]


def prep_in_maps(inputs: dict) -> list:
    """Host-side staging: cast weights/activations to bf16, pre-transpose x."""
    x = np.asarray(inputs["x"], dtype=np.float32)
    w1 = np.asarray(inputs["w1"], dtype=np.float32)
    b1 = np.asarray(inputs["b1"], dtype=np.float32)
    w2 = np.asarray(inputs["w2"], dtype=np.float32)
    b2 = np.asarray(inputs["b2"], dtype=np.float32)
    in_maps = []
    for e in range(E):
        xT = np.ascontiguousarray(x[:, e].reshape(TOK, D).T).astype(ml_dtypes.bfloat16)
        in_maps.append(
            {
                "xT": xT,
                "w1": np.ascontiguousarray(w1[e]).astype(ml_dtypes.bfloat16),
                "b1": np.ascontiguousarray(b1[e]),
                "w2": np.ascontiguousarray(w2[e]).astype(ml_dtypes.bfloat16),
                "b2": np.ascontiguousarray(b2[e]),
            }
        )
    return in_maps


def kernel(x, w1, b1, w2, b2):
    from concourse.bass_utils import run_bass_kernel_spmd

    nc = _get_nc()
    in_maps = prep_in_maps({"x": x, "w1": w1, "b1": b1, "w2": w2, "b2": b2})
    res = run_bass_kernel_spmd(nc, in_maps, list(range(E)))
    out = np.empty((B, E, N, D), np.float32)
    for e in range(E):
        out[:, e] = res.results[e]["out"].reshape(B, N, D)
    return out
